# revision 1
# baseline (speedup 1.0000x reference)
"""AGNNConv on 8 TRN2 NeuronCores — pure-compute streaming design, v3.

This platform (axon/PJRT TRN2) has no usable data-dependent DMA: the
custom SWDGE gather/scatter ucode crashes the device and the generic
indirect DMA path is a ~66us/call software queue.  So the kernel is
built exclusively from streaming DMA + compute engines:

  - Host (pure layout, no numerics): assign nodes to 32-slot windows
    with a greedy in-degree balancer + repair pass so every window
    carries <= 512 edges (tpw=4 tiles of 128 edge slots — ~0.4% pad).
    Partition edges by dst window (392 windows per core), and
    materialize per-edge operand rows fs = feat[src], fd = feat[dst]
    plus a 32-wide one-hot of slot(dst) in the exact SBUF layout the
    device consumes.
  - Device per batch of 28 windows (engine assignment chosen against
    the cost model: gpsimd runs tensor ops at 0.42 efficiency; DVE
    add-trees beat 1x tensor_reduce; ACT uses only Square/Exp/Copy,
    all in one activation-table set, so no table reloads; the rsqrt
    is a DVE fast-inverse-sqrt + 2 Newton steps, avoiding Ln):
      ACT:  sq = fs^2, sq2 = fd^2      (Square)
      DVE:  prod = fs*fd (bf16 2x), 3 add-trees -> ss_s, ss_d, cos
      DVE:  rn = rsqrt(ss_s*ss_d + eps) via bit-hack + Newton
      ACT:  p = exp(beta * cos * rn)
      POOL: payload = [p*fs | p] bf16  (single remaining gpsimd mul)
      PE:   scatter — per 128-edge tile a one-hot A[e, m] fp8 and
            A^T @ payload accumulates [32 nodes, 33] into a quadrant
            (tile_position) of a shared PSUM tile; 4 windows per PSUM
            tile, accumulated across each window's 4 tiles.
      out = msg / s on evacuation.
  - Pad edge slots have all-zero fs/fd/one-hot rows: they stay finite
    through the norm chain and contribute nothing to the scatter.
"""

import sys

if "/opt/trn_rl_repo" not in sys.path:
    sys.path.insert(0, "/opt/trn_rl_repo")

import os as _os

import numpy as np

# Problem constants (hardcoded per harness contract)
N_NODES = 100000
N_EDGES = 1600000
D = 32
NCORES = 8
WSZ = 64           # dst window size (one-hot width)
NW = 196           # windows per core
NWIN = NCORES * NW # 1568 global windows
TPW = 8            # tiles (128 edge slots) per window (balanced bins)
WB = int(_os.environ.get("WB", "14"))  # windows per batch, even
PW = D + 1         # payload width
MAGIC = 0x5F3759DF # fast inverse sqrt seed
NEWTON_ITERS = 1   # rsqrt Newton refinements (1 -> ~0.17% max rel err)
PSG = 4            # PSUM quad-groups per tile (fewer ACT evacuations)
BUFS_INP = int(_os.environ.get("BUFS_INP", "3"))
BUFS_MED = int(_os.environ.get("BUFS_MED", "2"))
BUFS_SML = int(_os.environ.get("BUFS_SML", "4"))


def build_graph(nw, tpw, wb, wsz=WSZ, d=D, repeat=1,
                skip_pe=False, skip_a=False, skip_norm=False, skip_cos=False,
                skip_pay=False):
    import concourse.bass as bass
    import concourse.tile as tile
    from concourse import bacc, mybir
    from contextlib import nullcontext

    f32 = mybir.dt.float32
    bf16 = mybir.dt.bfloat16
    fp8 = mybir.dt.float8e4
    u32 = mybir.dt.uint32
    SQ = mybir.ActivationFunctionType.Square
    EXP = mybir.ActivationFunctionType.Exp
    MULT = mybir.AluOpType.mult
    ADD = mybir.AluOpType.add
    SUB = mybir.AluOpType.subtract
    SHR = mybir.AluOpType.logical_shift_right

    assert nw % wb == 0 and wb % 2 == 0
    nb = nw // wb
    tb = wb * tpw   # tiles per batch
    qb = wb // 2    # PSUM pair-tiles per batch (2 windows each)
    nquad = nw // 2

    nc = bacc.Bacc(None, target_bir_lowering=False, debug=False)
    fs_p = nc.declare_dram_parameter("fs", [128, nw, tpw, d], bf16, isOutput=False)
    fd_p = nc.declare_dram_parameter("fd", [128, nw, tpw, d], bf16, isOutput=False)
    ah_p = nc.declare_dram_parameter("ah", [128, nw, tpw, wsz], fp8, isOutput=False)
    beta_p = nc.declare_dram_parameter("beta", [1], f32, isOutput=False)
    out_p = nc.declare_dram_parameter("out", [nw * wsz, d], f32, isOutput=True)

    outR = out_p[:].rearrange("(j m) c -> m j c", m=128)

    def tree_sum(pool, src, out, eps=0.0):
        """out[128, tb, 1] f32 = row sums over last dim (32) of bf16 src
        (+ eps, fused into the last level)."""
        t1 = pool.tile([128, tb, 16], bf16)
        nc.vector.tensor_add(t1[:], src[:, :, 0:16], src[:, :, 16:32])
        t2 = pool.tile([128, tb, 8], bf16)
        nc.vector.tensor_add(t2[:], t1[:, :, 0:8], t1[:, :, 8:16])
        t3 = pool.tile([128, tb, 4], bf16)
        nc.vector.tensor_add(t3[:], t2[:, :, 0:4], t2[:, :, 4:8])
        t4 = pool.tile([128, tb, 2], bf16)
        nc.vector.tensor_add(t4[:], t3[:, :, 0:2], t3[:, :, 2:4])
        if eps:
            nc.vector.scalar_tensor_tensor(
                out=out[:], in0=t4[:, :, 0:1], scalar=eps,
                in1=t4[:, :, 1:2], op0=ADD, op1=ADD,
            )
        else:
            nc.vector.tensor_add(out[:], t4[:, :, 0:1], t4[:, :, 1:2])

    with tile.TileContext(nc) as tc:
        with tc.tile_pool(name="singles", bufs=1) as singles:
            beta_sb = singles.tile([128, 1], f32)
            nc.sync.dma_start(out=beta_sb[:], in_=beta_p[:].to_broadcast([128, 1]))
            magic_sb = singles.tile([128, 1], u32)
            nc.vector.memset(magic_sb[:], MAGIC)
            obuf = singles.tile([128, nquad, d], f32)
            stg_all = singles.tile([128, nquad, PW], f32)
            if skip_pe:
                nc.vector.memset(obuf[:], 0.0)
            A_pers = cn_pers = pay_pers = None
            if skip_a:
                A_pers = singles.tile([128, wb, tpw, wsz], fp8)
                nc.vector.memset(A_pers[:], 0.0)
            if skip_cos and skip_norm:
                cn_pers = singles.tile([128, tb, 1], f32)
                nc.vector.memset(cn_pers[:], 0.5)
            if skip_pay:
                pay_pers = singles.tile([128, tb, PW], bf16)
                nc.vector.memset(pay_pers[:], 0.0)

            with (
                tc.tile_pool(name="inp", bufs=BUFS_INP) as inp,
                tc.tile_pool(name="ap_", bufs=BUFS_INP) as ap_,
                tc.tile_pool(name="med", bufs=BUFS_MED) as med,
                tc.tile_pool(name="sml", bufs=BUFS_SML) as sml,
                tc.tile_pool(name="ps_", bufs=4, space="PSUM") as ps_,
                tc.For_i(0, repeat, 1) if repeat > 1 else nullcontext(),
            ):
                pending = []  # deferred PSUM evacuations (sw pipelining)

                def flush_evac():
                    while pending:
                        ps_tile, q0, ng = pending.pop(0)
                        nc.scalar.copy(
                            out=stg_all[:, q0 : q0 + ng, :], in_=ps_tile[:]
                        )

                for b in range(nb):
                    ws = slice(b * wb, (b + 1) * wb)
                    fs_t = inp.tile([128, wb, tpw, d], bf16)
                    nc.sync.dma_start(out=fs_t[:], in_=fs_p[:, ws, :, :])
                    fd_t = inp.tile([128, wb, tpw, d], bf16)
                    nc.scalar.dma_start(out=fd_t[:], in_=fd_p[:, ws, :, :])
                    if not skip_a:
                        A_t = ap_.tile([128, wb, tpw, wsz], fp8)
                        nc.sync.dma_start(out=A_t[:], in_=ah_p[:, ws, :, :])
                        A_f = A_t[:].rearrange("i w t m -> i (w t) m")
                    else:
                        A_f = A_pers[:].rearrange("i w t m -> i (w t) m")

                    fsf = fs_t[:].rearrange("i w t c -> i (w t) c")
                    fdf = fd_t[:].rearrange("i w t c -> i (w t) c")

                    # ACT squares go first in the ACT queue so the next
                    # batch's DVE trees never wait behind this batch's
                    # exp (which depends on the DVE chain)
                    if not skip_norm:
                        sq = med.tile([128, tb, d], bf16)
                        nc.scalar.activation(sq[:], fsf, SQ)
                        sq2 = med.tile([128, tb, d], bf16)
                        nc.scalar.activation(sq2[:], fdf, SQ)
                    cn = sml.tile([128, tb, 1], f32)
                    if not skip_cos:
                        prod = med.tile([128, tb, d], bf16)
                        nc.vector.tensor_mul(prod[:], fsf, fdf)
                        cos = sml.tile([128, tb, 1], f32)
                        tree_sum(med, prod[:], cos)
                    if not skip_norm:
                        ss_s = sml.tile([128, tb, 1], f32)
                        tree_sum(med, sq[:], ss_s, eps=1e-18)
                        ss_d = sml.tile([128, tb, 1], f32)
                        tree_sum(med, sq2[:], ss_d, eps=1e-18)

                        # x = ss_s*ss_d; rn = rsqrt(x) via bit hack +
                        # Newton step(s) (no ACT table needed)
                        x = sml.tile([128, tb, 1], f32)
                        nc.vector.tensor_mul(x[:], ss_s[:], ss_d[:])
                        yi = sml.tile([128, tb, 1], u32)
                        nc.vector.tensor_scalar(
                            out=yi[:], in0=x[:].bitcast(u32), scalar1=1,
                            scalar2=None, op0=SHR,
                        )
                        y0 = sml.tile([128, tb, 1], u32)
                        nc.vector.tensor_tensor(
                            out=y0[:], in0=magic_sb[:].to_broadcast([128, tb, 1]),
                            in1=yi[:], op=SUB,
                        )
                        ya = y0[:].bitcast(f32)
                        rn = sml.tile([128, tb, 1], f32)
                        for it in range(NEWTON_ITERS):
                            # y <- y*(1.5 - 0.5*x*y^2), 3 fused DVE ops
                            t1 = sml.tile([128, tb, 1], f32)
                            nc.vector.tensor_mul(t1[:], ya, ya)
                            u1 = sml.tile([128, tb, 1], f32)
                            nc.vector.scalar_tensor_tensor(
                                out=u1[:], in0=t1[:], scalar=-0.5, in1=x[:],
                                op0=MULT, op1=MULT,
                            )
                            if it == NEWTON_ITERS - 1:
                                dst = rn
                            else:
                                dst = sml.tile([128, tb, 1], f32)
                            nc.vector.scalar_tensor_tensor(
                                out=dst[:], in0=u1[:], scalar=1.5, in1=ya,
                                op0=ADD, op1=MULT,
                            )
                            ya = dst[:]
                        if not skip_cos:
                            nc.vector.tensor_mul(cn[:], cos[:], rn[:])
                        else:
                            nc.vector.tensor_copy(out=cn[:], in_=rn[:])
                    elif not skip_cos:
                        nc.vector.tensor_copy(out=cn[:], in_=cos[:])
                    else:
                        cn = cn_pers

                    # p = exp(beta*cn)
                    p_t = sml.tile([128, tb, 1], bf16)
                    nc.scalar.activation(p_t[:], cn[:], EXP, scale=beta_sb[:])

                    # payload [p*fs | p] in bf16 (POOL's single big mul)
                    if not skip_pay:
                        pay = med.tile([128, tb, PW], bf16)
                        nc.gpsimd.tensor_mul(
                            pay[:, :, 0:d], fsf, p_t[:].to_broadcast([128, tb, d])
                        )
                        nc.scalar.copy(out=pay[:, :, d : d + 1], in_=p_t[:])
                    else:
                        pay = pay_pers

                    # scatter: PSUM accumulation; 4 windows per quadrant
                    # column, PSG quad-groups per PSUM tile.  Evacuation
                    # is deferred one batch (sw pipelining) so the ACT
                    # queue never stalls on PE completion; divisions all
                    # happen once after the loop.
                    if not skip_pe:
                        for pg in range(0, qb, PSG):
                            ng = min(PSG, qb - pg)
                            ps = ps_.tile([128, ng, PW], f32)
                            for g in range(ng):
                                pj = pg + g
                                for q in range(2):
                                    wj = pj * 2 + q
                                    for t_i in range(tpw):
                                        ti = wj * tpw + t_i
                                        nc.tensor.matmul(
                                            ps[q * wsz : (q + 1) * wsz, g, :],
                                            lhsT=A_f[:, ti, :],
                                            rhs=pay[:, ti, :],
                                            start=(t_i == 0),
                                            stop=(t_i == tpw - 1),
                                        )
                            pending.append((ps, b * qb + pg, ng))

                    # evacuate the PREVIOUS batch's PSUM tiles here: its
                    # PE work finished during this batch's compute, so
                    # the ACT queue never blocks on it
                    if not skip_pe and b > 0:
                        while len(pending) > 2:
                            ps_tile, q0, ng = pending.pop(0)
                            nc.scalar.copy(
                                out=stg_all[:, q0 : q0 + ng, :], in_=ps_tile[:]
                            )

                if not skip_pe:
                    flush_evac()
                    # tail: out = msg / max(s, eps), one pass over all
                    # windows
                    scb = sml.tile([128, nquad, 1], f32)
                    nc.vector.tensor_scalar_max(
                        scb[:], stg_all[:, :, d : d + 1], 1e-30
                    )
                    rcb = sml.tile([128, nquad, 1], f32)
                    nc.vector.reciprocal(rcb[:], scb[:])
                    nc.vector.tensor_mul(
                        obuf[:, :, :],
                        stg_all[:, :, 0:d],
                        rcb[:].to_broadcast([128, nquad, d]),
                    )

            nc.sync.dma_start(out=outR[:, :, :], in_=obuf[:])

    nc.compile()
    return nc


def _balance_windows(deg, nwin, wsz, cap):
    """Greedy LPT bin packing (desc in-degree) into nwin windows of wsz
    node slots, then a repair pass to push per-window edge counts under
    cap.  Returns (win_of, slot_of)."""
    import heapq

    n = deg.size
    win_of = np.empty(n, np.int64)
    fill = np.zeros(nwin, np.int64)
    load = np.zeros(nwin, np.int64)
    members = [[] for _ in range(nwin)]
    order = np.argsort(-deg, kind="stable")
    heap = [(0, w) for w in range(nwin)]
    heapq.heapify(heap)
    deg_l = deg.tolist()
    for idx in order.tolist():
        while True:
            e, w = heapq.heappop(heap)
            if fill[w] < wsz:
                break
        win_of[idx] = w
        members[w].append(idx)
        fill[w] += 1
        load[w] = e + deg_l[idx]
        if fill[w] < wsz:
            heapq.heappush(heap, (load[w], w))

    # repair: swap nodes from over-cap windows with lighter nodes from
    # under-cap windows until every window fits (best-effort)
    over = [w for w in range(nwin) if load[w] > cap]
    if over:
        under = sorted(
            (w for w in range(nwin) if load[w] < cap),
            key=lambda w: load[w],
        )
        ui = 0
        for w in over:
            members[w].sort(key=lambda i: deg_l[i])
            while load[w] > cap and ui < len(under):
                need = load[w] - cap
                uw = under[ui]
                slack = cap - load[uw]
                if slack <= 0:
                    ui += 1
                    continue
                # pick the node in w whose degree best uses the slack
                best = None
                for i in members[w]:
                    dd = deg_l[i]
                    if dd == 0:
                        continue
                    # swap candidate from uw: smallest-degree node
                    j = min(members[uw], key=lambda k: deg_l[k])
                    gain = dd - deg_l[j]
                    if gain >= need and gain <= slack:
                        best = (i, j)
                        break
                    if gain > 0 and gain <= slack and best is None:
                        best = (i, j)
                if best is None:
                    ui += 1
                    continue
                i, j = best
                members[w].remove(i)
                members[uw].remove(j)
                members[w].append(j)
                members[uw].append(i)
                load[w] += deg_l[j] - deg_l[i]
                load[uw] += deg_l[i] - deg_l[j]
                win_of[i], win_of[j] = uw, w
                if load[uw] >= cap:
                    ui += 1

    slot_of = np.empty(n, np.int64)
    cnt = np.zeros(nwin, np.int64)
    for idx in range(n):
        w = win_of[idx]
        slot_of[idx] = cnt[w]
        cnt[w] += 1
    return win_of, slot_of


def host_prep(feat, beta, src, dst, ncores=NCORES, nw=NW, d=D, wsz=WSZ):
    """Pure index/layout prep. Returns (per-core input maps, tpw, pos_of)."""
    import ml_dtypes

    feat = np.ascontiguousarray(np.asarray(feat, dtype=np.float32))
    beta = np.ascontiguousarray(np.asarray(beta, dtype=np.float32))
    src = np.asarray(src).astype(np.int64)
    dst = np.asarray(dst).astype(np.int64)
    nwin = ncores * nw
    n_nodes = feat.shape[0]

    deg = np.bincount(dst, minlength=n_nodes)
    win_of, slot_of = _balance_windows(deg, nwin, wsz, TPW * 128)

    ewin = win_of[dst]
    order = np.argsort(ewin, kind="stable")
    src_s, dst_s, win_s = src[order], dst[order], ewin[order]
    wcnt = np.bincount(win_s, minlength=nwin)
    tpw = max(TPW, int(-(-int(wcnt.max()) // 128)))
    starts = np.concatenate([[0], np.cumsum(wcnt)[:-1]])
    rank = np.arange(src_s.size) - starts[win_s]
    t_all = rank // 128
    i_all = rank % 128

    feat_bf = feat.astype(ml_dtypes.bfloat16)
    eye = np.eye(wsz, dtype=ml_dtypes.float8_e4m3fn)
    slot_e = slot_of[dst_s]

    in_maps = []
    for c in range(ncores):
        lo_w, hi_w = c * nw, (c + 1) * nw
        sel = (win_s >= lo_w) & (win_s < hi_w)
        e_src, e_win, e_slot, t_, i_ = (
            src_s[sel], win_s[sel] - lo_w, slot_e[sel], t_all[sel], i_all[sel],
        )
        e_dst = dst_s[sel]

        fs = np.zeros((128, nw, tpw, d), dtype=ml_dtypes.bfloat16)
        fd = np.zeros((128, nw, tpw, d), dtype=ml_dtypes.bfloat16)
        ah = np.zeros((128, nw, tpw, wsz), dtype=ml_dtypes.float8_e4m3fn)
        fs[i_, e_win, t_] = feat_bf[e_src]
        fd[i_, e_win, t_] = feat_bf[e_dst]
        ah[i_, e_win, t_] = eye[e_slot]

        in_maps.append({"fs": fs, "fd": fd, "ah": ah, "beta": beta})

    pos_of = win_of * wsz + slot_of  # global output row of each node
    return in_maps, tpw, pos_of


_CACHED = {}


def kernel(feat, beta, src, dst):
    from concourse.bass_utils import run_bass_kernel_spmd

    in_maps, tpw, pos_of = host_prep(feat, beta, src, dst)
    key = ("nc", tpw)
    if key not in _CACHED:
        _CACHED[key] = build_graph(NW, tpw, WB)
    nc = _CACHED[key]
    res = run_bass_kernel_spmd(nc, in_maps, list(range(NCORES))).results
    full = np.concatenate([res[c]["out"] for c in range(NCORES)], axis=0)
    return full[pos_of].astype(np.float32)

